# revision 4
# baseline (speedup 1.0000x reference)
"""Trainium2 Bass kernel for nn_Attention_7962869366891.

Module: y = x + Wo @ attn(LN_q(x) Wq, LN_c(x) Wkv)   with B=4, N=2048, F=1024,
H=16 heads, D=64.

Sharding (8 cores): core c -> (batch b = c//2, head-group g = c%2 of 8 heads).
Each core computes a full [N, F] partial of the output projection for its 8
heads; the host sums the two partials per batch plus the residual skip.

Device-side design (per core), v4:
  - bf16 datapath, fp32 PSUM/stats/normalization.
  - x feature-major in 32 [128,512] tiles, chunk-major stats so LN/z/Q0/K0
    start while later-chunk stats still accumulate.  x^2 for the variance
    comes from ACT (Square) -- the Vector engine is the lead-in bottleneck.
  - LN affine folded into weights host-side; per-token -mu*rstd rides as one
    K=2 matmul per accumulation group.
  - Attention blocks of (head, nhalf) x 16 key-tiles: St = k^T q, exp from
    PSUM into bf16 pt, O^T = V'^T P with a ones column for the denominator.
  - Block order: all nhalf=0 blocks for pairs 0..3, then all nhalf=1.  After
    the nh0 round, out-proj chunks 0,1 unlock and feed the nh1 round's PE.
  - ScalarE exp (~295us) is the floor; PE work exceeds it, so PE must stay
    dense (HAM clock-gate): an ordered filler inventory (V proj, later
    pairs' Q/K, out-proj) is emitted inside the m-loops at fixed slots and
    at every block boundary.
  - o_ps PSUM slots released fast via DVE copies; the reciprocal broadcast
    (gpsimd DRAM bounce) runs off-critical-path.
"""

import numpy as np
import ml_dtypes

import concourse.bass as bass
import concourse.bacc as bacc
import concourse.mybir as mybir
import concourse.tile as tile
from concourse.bass_utils import run_bass_kernel_spmd

F32 = mybir.dt.float32
BF16 = mybir.dt.bfloat16
AF = mybir.ActivationFunctionType

B, N, F, H, D = 4, 2048, 1024, 16, 64
HG = 8                # heads per core
E = HG * D            # 512 projection dims per core
NT = N // 128         # 16 token tiles
FTC = F // 128        # 8 feature tiles
ET = E // 128         # 4 e-tiles (head pairs)
NCH = N // 512        # 4 token chunks of 512
EPS = 1e-5

_CACHE = {}


def build_program():
    nc = bacc.Bacc("TRN2", target_bir_lowering=False, debug=False, num_devices=8)

    xT = nc.dram_tensor("xT", [F, N], BF16, kind="ExternalInput").ap()
    wq = nc.dram_tensor("wq", [F, E], BF16, kind="ExternalInput").ap()
    wk = nc.dram_tensor("wk", [F, E], BF16, kind="ExternalInput").ap()
    wv = nc.dram_tensor("wv", [F, E], BF16, kind="ExternalInput").ap()
    corr = nc.dram_tensor("corr", [2, 3 * E], BF16, kind="ExternalInput").ap()
    wo = nc.dram_tensor("wo", [E, F], BF16, kind="ExternalInput").ap()
    onesd = nc.dram_tensor("onesd", [128, 512], BF16, kind="ExternalInput").ap()
    out = nc.dram_tensor("out", [N, F], BF16, kind="ExternalOutput").ap()
    scr = nc.dram_tensor("scr", [HG * NCH, 512], F32).ap()

    with tile.TileContext(nc) as tc:
        _emit(nc, tc, xT, wq, wk, wv, corr, wo, onesd, out, scr)
    nc.compile()
    return nc


def _emit(nc, tc, xT, wq, wk, wv, corr, wo, onesd, out, scr):
    from contextlib import ExitStack
    pers = ExitStack()
    with pers:
        # ---------------- persistent constants ----------------
        single = pers.enter_context(tc.tile_pool(name="single", bufs=1))
        ones128 = single.tile([128, 128], BF16)
        nc.sync.dma_start(out=ones128, in_=onesd[:, 0:128])
        zero_c = single.tile([128, 1], F32)
        nc.vector.memset(zero_c, 0.0)
        eps_c = single.tile([128, 1], F32)
        nc.vector.memset(eps_c, EPS)
        # per-chunk [mu*rstd; ones] rows for the K=2 correction matmul
        aug = [single.tile([2, 512], BF16, name=f"aug_{c}", tag=f"aug_{c}")
               for c in range(NCH)]
        for c in range(NCH):
            nc.sync.dma_start(out=aug[c][1:2, :], in_=onesd[0:1, :])
        corr2 = single.tile([2, 3 * E], BF16)  # row0 = -rowsum(W'), row1 = bias
        nc.sync.dma_start(out=corr2, in_=corr)

        # ---------------- x (32 fine tiles, chunk-major) ----------------
        xpool = pers.enter_context(tc.tile_pool(name="x", bufs=1))
        xt = [[None] * NCH for _ in range(FTC)]
        for c in range(NCH):
            for ft in range(FTC):
                t = xpool.tile([128, 512], BF16, name=f"x_{ft}_{c}",
                               tag=f"x_{ft}_{c}")
                nc.sync.dma_start(
                    out=t,
                    in_=xT[ft * 128:(ft + 1) * 128, c * 512:(c + 1) * 512])
                xt[ft][c] = t

        # ---------------- weights ----------------
        wpool = pers.enter_context(tc.tile_pool(name="w", bufs=1))
        wq_sb, wk_sb, wv_sb = [], [], []
        for wdram, lst, nm in ((wq, wq_sb, "wq"), (wk, wk_sb, "wk"),
                               (wv, wv_sb, "wv")):
            for ft in range(FTC):
                t = wpool.tile([128, E], BF16, name=f"{nm}_{ft}",
                               tag=f"{nm}_{ft}")
                nc.sync.dma_start(out=t, in_=wdram[ft * 128:(ft + 1) * 128, :])
                lst.append(t)
        wo_sb = []
        for et in range(ET):
            t = wpool.tile([128, F], BF16, name=f"wo_{et}", tag=f"wo_{et}")
            nc.sync.dma_start(out=t, in_=wo[et * 128:(et + 1) * 128, :])
            wo_sb.append(t)

        # ---------------- LN stats (chunk-major) ----------------
        rp = pers.enter_context(tc.tile_pool(name="rp", bufs=1))
        rb = [rp.tile([128, 512], F32, name=f"rb_{c}", tag=f"rb_{c}")
              for c in range(NCH)]
        with tc.tile_pool(name="pstats", bufs=1, space="PSUM") as pstats, \
             tc.tile_pool(name="xsq", bufs=2) as xsqp, \
             tc.tile_pool(name="statf", bufs=4) as statf:
            for c in range(NCH):
                s1 = pstats.tile([128, 512], F32, tag=f"s1{c}", name=f"s1_{c}")
                s2 = pstats.tile([128, 512], F32, tag=f"s2{c}", name=f"s2_{c}")
                for ft in range(FTC):
                    xs = xsqp.tile([128, 512], BF16, tag="xsq")
                    nc.scalar.activation(xs, xt[ft][c], AF.Square,
                                         bias=zero_c)
                    nc.tensor.matmul(s1, ones128, xt[ft][c],
                                     start=(ft == 0), stop=(ft == FTC - 1))
                    nc.tensor.matmul(s2, ones128, xs,
                                     start=(ft == 0), stop=(ft == FTC - 1))
                mu = statf.tile([128, 512], F32, tag="mu", name=f"mu_{c}")
                va = statf.tile([128, 512], F32, tag="va", name=f"va_{c}")
                m2 = statf.tile([128, 512], F32, tag="m2", name=f"m2_{c}")
                nc.vector.tensor_scalar_mul(mu, s1, 1.0 / F)
                nc.vector.tensor_scalar_mul(va, s2, 1.0 / F)
                nc.vector.tensor_mul(m2, mu, mu)
                nc.vector.tensor_sub(va, va, m2)   # var
                nc.scalar.activation(m2, va, AF.Ln, bias=eps_c)
                nc.scalar.activation(rb[c], m2, AF.Exp,
                                     bias=zero_c, scale=-0.5)
                nc.vector.tensor_mul(mu, mu, rb[c])            # mu*rstd
                nc.vector.tensor_copy(aug[c][0:1, :], mu[0:1, :])
                for ft in range(FTC):                          # z = x*rstd
                    nc.vector.tensor_mul(xt[ft][c], xt[ft][c], rb[c])

        # ---------------- result tiles ----------------
        qkpool = pers.enter_context(tc.tile_pool(name="qk", bufs=1, side="right"))
        qt = [qkpool.tile([128, N], BF16, name=f"qt_{et}", tag=f"qt_{et}")
              for et in range(ET)]
        kt = [qkpool.tile([128, N], BF16, name=f"kt_{et}", tag=f"kt_{et}")
              for et in range(ET)]
        vpool = pers.enter_context(tc.tile_pool(name="vtok", bufs=1, side="right"))
        vt = [vpool.tile([128, HG * (D + 1)], BF16, name=f"vt_{m}",
                         tag=f"vt_{m}") for m in range(NT)]
        vt_r = [t.rearrange("p (h x) -> p h x", x=D + 1) for t in vt]
        opool = pers.enter_context(tc.tile_pool(name="ostk", bufs=1, side="right"))
        ot = [[opool.tile([128, 512], BF16, name=f"ot_{et}_{c}",
                          tag=f"ot_{et}_{c}") for c in range(NCH)]
              for et in range(ET)]
        obp = pers.enter_context(tc.tile_pool(name="obp", bufs=4))

        def qk_group(pool, wsb, wi, et, c, dest):
            def emit():
                crA = corr2[:, wi * E + et * 128: wi * E + (et + 1) * 128]
                ps = pool.tile([128, 512], F32, tag="pp",
                               name=f"pp{wi}_{et}_{c}")
                for ft in range(FTC):
                    nc.tensor.matmul(ps, wsb[ft][:, et * 128:(et + 1) * 128],
                                     xt[ft][c], start=(ft == 0), stop=False)
                nc.tensor.matmul(ps, crA, aug[c], start=False, stop=True)
                nc.vector.tensor_copy(dest[et][:, c * 512:(c + 1) * 512], ps)
            return emit

        def v_group(pool, m):
            def emit():
                c, js = m // 4, slice((m % 4) * 128, (m % 4 + 1) * 128)
                nc.sync.dma_start(out=vt_r[m][:, :, D:D + 1],
                                  in_=onesd[:, 0:HG])
                ps = pool.tile([128, 512], F32, tag="pp", name=f"ppv_{m}")
                for ft in range(FTC):
                    nc.tensor.matmul(ps, xt[ft][c][:, js], wv_sb[ft],
                                     start=(ft == 0), stop=False)
                nc.tensor.matmul(ps, aug[c][:, js], corr2[:, 2 * E:3 * E],
                                 start=False, stop=True)
                nc.vector.tensor_copy(
                    vt_r[m][:, :, 0:D],
                    ps.rearrange("p (h d) -> p h d", d=D))
            return emit

        def outproj_group(pool, tt, fc):
            def emit():
                c = tt // 4
                js = slice((tt % 4) * 128, (tt % 4 + 1) * 128)
                ts_ = slice(tt * 128, (tt + 1) * 128)
                fs = slice(fc * 512, (fc + 1) * 512)
                ps = pool.tile([128, 512], F32, tag="pp",
                               name=f"pso{tt}_{fc}")
                for et in range(ET):
                    nc.tensor.matmul(ps, ot[et][c][:, js], wo_sb[et][:, fs],
                                     start=(et == 0), stop=(et == ET - 1))
                ob = obp.tile([128, 512], BF16, tag="ob", name=f"ob{tt}_{fc}")
                nc.vector.tensor_copy(ob, ps)
                nc.sync.dma_start(out=out[ts_, fs], in_=ob)
            return emit

        qkv = ExitStack()
        with qkv:
            pst = qkv.enter_context(tc.tile_pool(name="pst", bufs=2, space="PSUM"))
            po = qkv.enter_context(tc.tile_pool(name="po", bufs=2, space="PSUM"))
            pwork = qkv.enter_context(tc.tile_pool(name="pwork", bufs=2,
                                                   space="PSUM"))
            ptp = qkv.enter_context(tc.tile_pool(name="ptp", bufs=3))
            dnp = qkv.enter_context(tc.tile_pool(name="dn", bufs=4))
            oup = qkv.enter_context(tc.tile_pool(name="ou", bufs=4))

            filler = []
            fidx = [0]

            def emit_filler(n=1):
                while n > 0 and fidx[0] < len(filler):
                    filler[fidx[0]]()
                    fidx[0] += 1
                    n -= 1

            def attn_block(p, h, nh, emit_at=(1, 4, 7, 10, 13)):
                er = (h % 2) * 64
                o_ps = [po.tile([65, 512], F32, tag="ops",
                                name=f"ops{h}_{nh}_{i}") for i in range(2)]
                for m in range(NT):
                    ms_ = slice(m * 128, (m + 1) * 128)
                    st = pst.tile([128, 1024], F32, tag="st",
                                  name=f"st{h}_{nh}_{m}")
                    for i in range(2):
                        c = 2 * nh + i
                        cs = slice(c * 512, (c + 1) * 512)
                        nc.tensor.matmul(st[:, i * 512:(i + 1) * 512],
                                         kt[p][er:er + 64, ms_],
                                         qt[p][er:er + 64, cs],
                                         start=True, stop=True)
                    pt = ptp.tile([128, 1024], BF16, tag="pt",
                                  name=f"pt{h}_{nh}_{m}")
                    nc.scalar.activation(pt, st, AF.Exp, bias=zero_c)
                    for i in range(2):
                        nc.tensor.matmul(o_ps[i], vt_r[m][:, h, :],
                                         pt[:, i * 512:(i + 1) * 512],
                                         start=(m == 0), stop=(m == NT - 1))
                    if m in emit_at:
                        emit_filler(1)
                for i in range(2):
                    c = 2 * nh + i
                    rr = dnp.tile([1, 512], F32, tag="rr", name=f"rr{h}_{c}")
                    nc.vector.reciprocal(rr, o_ps[i][64:65, :])
                    ou = oup.tile([64, 512], F32, tag="ou", name=f"ou{h}_{c}")
                    nc.vector.tensor_copy(ou, o_ps[i][0:64, :])
                    rbt = dnp.tile([64, 512], F32, tag="rb", name=f"rbt{h}_{c}")
                    sr = scr[h * NCH + c:h * NCH + c + 1, :]
                    nc.gpsimd.dma_start(out=sr, in_=rr)
                    nc.gpsimd.dma_start(out=rbt, in_=sr.to_broadcast([64, 512]))
                    nc.vector.tensor_mul(ot[p][c][er:er + 64, :], ou, rbt)
                emit_filler(1)   # boundary filler keeps PE fed across blocks

            # -------- upfront: K0 (all), Q0 c0/c1, V m0-3 --------
            for c in range(NCH):
                qk_group(pwork, wk_sb, 1, 0, c, kt)()
            for c in range(2):
                qk_group(pwork, wq_sb, 0, 0, c, qt)()
            for m in range(4):
                v_group(pwork, m)()

            # -------- ordered filler inventory --------
            filler += [v_group(pwork, m) for m in range(4, NT)]
            for p in (1, 2, 3):
                filler += [qk_group(pwork, wk_sb, 1, p, c, kt)
                           for c in range(2)]
                filler += [qk_group(pwork, wq_sb, 0, p, c, qt)
                           for c in range(2)]
                filler += [qk_group(pwork, wk_sb, 1, p, c, kt)
                           for c in (2, 3)]
            for p in (0, 1, 2, 3):
                filler += [qk_group(pwork, wq_sb, 0, p, c, qt)
                           for c in (2, 3)]

            # -------- nh0 round --------
            attn_block(0, 0, 0, emit_at=tuple(range(12)))   # V rides here
            attn_block(0, 1, 0)
            for p in (1, 2, 3):
                attn_block(p, 2 * p, 0)
                attn_block(p, 2 * p + 1, 0)
            # out-proj chunks 0,1 now unlocked
            filler += [outproj_group(pwork, tt, fc)
                       for tt in range(8) for fc in range(2)]

            # -------- nh1 round --------
            for p in (0, 1, 2, 3):
                attn_block(p, 2 * p, 1)
                attn_block(p, 2 * p + 1, 1)
            emit_filler(len(filler))

        # -------- tail: out-proj chunks 2,3 (deep PSUM pipeline) --------
        with tc.tile_pool(name="ptail", bufs=4, space="PSUM") as ptail:
            for tt in range(8, NT):
                for fc in range(2):
                    outproj_group(ptail, tt, fc)()


def _prep(inputs):
    x = np.asarray(inputs["x"], np.float32)
    Wq = np.asarray(inputs["Wq"], np.float32)
    Wkv = np.asarray(inputs["Wkv"], np.float32)
    Wo = np.asarray(inputs["Wo"], np.float32)
    ln_g = np.asarray(inputs["ln_g"], np.float32)
    ln_b = np.asarray(inputs["ln_b"], np.float32)
    lnc_g = np.asarray(inputs["lnc_g"], np.float32)
    lnc_b = np.asarray(inputs["lnc_b"], np.float32)

    bf = ml_dtypes.bfloat16
    qscale = np.float32(D ** -0.5)
    in_maps = []
    for c in range(8):
        b, g = c // 2, c % 2
        gs = slice(g * E, (g + 1) * E)
        Wq_g = Wq[gs] * ln_g[None, :] * qscale          # [E, F] (scale folded)
        cq = (Wq[gs] @ ln_b) * qscale                   # [E]
        Wk_g = Wkv[gs] * lnc_g[None, :]
        ck = Wkv[gs] @ lnc_b
        Wv_g = Wkv[H * D + g * E:H * D + (g + 1) * E] * lnc_g[None, :]
        cv = Wkv[H * D + g * E:H * D + (g + 1) * E] @ lnc_b
        corr = np.stack([
            np.concatenate([-Wq_g.sum(1), -Wk_g.sum(1), -Wv_g.sum(1)]),
            np.concatenate([cq, ck, cv]),
        ])                                              # [2, 3E]
        in_maps.append({
            "onesd": np.ones((128, 512), bf),
            "xT": np.ascontiguousarray(x[b].T).astype(bf),
            "wq": np.ascontiguousarray(Wq_g.T).astype(bf),
            "wk": np.ascontiguousarray(Wk_g.T).astype(bf),
            "wv": np.ascontiguousarray(Wv_g.T).astype(bf),
            "corr": np.ascontiguousarray(corr).astype(bf),
            "wo": np.ascontiguousarray(Wo[:, gs].T).astype(bf),
        })
    return in_maps


def kernel(**inputs):
    if "nc" not in _CACHE:
        _CACHE["nc"] = build_program()
    nc = _CACHE["nc"]
    in_maps = _prep(inputs)
    res = run_bass_kernel_spmd(nc, in_maps, list(range(8))).results
    x = np.asarray(inputs["x"], np.float32)
    out = np.empty((B, N, F), np.float32)
    for b in range(B):
        out[b] = (res[2 * b]["out"].astype(np.float32)
                  + res[2 * b + 1]["out"].astype(np.float32)
                  + x[b])
    return out


if __name__ == "__main__":
    import reference
    ins = {k: np.asarray(v) for k, v in reference.setup_inputs().items()}
    exp = np.asarray(reference.reference(**ins))
    got = kernel(**ins)
    err = np.abs(got - exp)
    rel = np.linalg.norm(got - exp) / np.linalg.norm(exp)
    print("max abs err:", err.max(), "rel:", rel)
